# revision 5
# baseline (speedup 1.0000x reference)
"""HAttentionNetwork Trainium2 kernel (v2).

Strategy (8 NeuronCores, data-parallel over bags):
- 4096 bags split into 80 contiguous chunks (10/core, <=64 bags each),
  balanced by sentence count; each chunk padded to Tc tiles of 128 sentences.
- Host pre-arranges per-core arrays so every device DMA is one big
  contiguous-row 2D slice per chunk (4 DMA instructions per chunk):
    xv [128, CH*Tc*256]  bf16   x values, tile-major (partition=sentence)
    xq [128, CH*2*Tc*128] bf16  x transposed halves (partition=hidden)
    mt [128, CH*2*Tc]    bf16   per-sentence (label, local-seg) scalars
- Per 128-sentence tile on device:
    fltT[s,c] = sum_h xq_h[h,s] * ccT_h[h,c]   (PE, 2 bf16 matmuls, [128,106])
    E = exp(fltT)  batched 4 tiles/op          (ACT, PSUM->SBUF bf16)
    et_l = sum_c (iota53==lbl) * E[:, 53l:53l+53]  (DVE scalar_tensor_tensor
                                                    with accum_out, x2)
    a2[:, 64l:64l+64] = (io64==sg) * et_l      (Pool tensor_scalar, x2)
    u2T_h += xv_j_h^T @ a2                     (PE, accumulates [hid, bagslot]
                                                -> no epilogue transpose)
    den_l += a2_l^T @ ones                     (PE, [64,1] x2)
- Chunk epilogue: rT = copy(u2T) (ACT), outp_l = rT_l @ dt_l (PE fp32),
  out = outp_0*inv(den_0) + outp_1*inv(den_1) + bias (DVE), DMA out.
Numerics: bf16 inputs, fp32 PSUM accumulation, fp32 disc projection
(same precision as the reference-validated baseline).
"""

import numpy as np

N_SENT = 262144
N_BAGS = 4096
HIDDEN = 256
L0 = 14
NCLS = 53
NCORE = 8
CHUNKS_PER_CORE = 10
NCHUNK = NCORE * CHUNKS_PER_CORE
MAX_BAGS_PER_CHUNK = 64
EXP_GROUP = 4

_CACHE = {}


def _patch_tile_drain():
    # This walrus build rejects Drain instructions carrying more than ~1 sync
    # wait. Split the Tile final-drain waits across SP nops, one wait each.
    import concourse.mybir as mybir
    import concourse.tile as tile_mod
    from concourse.vector_clock import ScopedClock

    if getattr(tile_mod.TileContext, "_drain_split_patched", False):
        return

    def _split_drain_and_barrier(self, tick_clock, wait_clock):
        drain_inst = self.nc.sync.drain()
        wait_clock.add_sem_waits(
            drain_inst.ins, ScopedClock({None: tick_clock.global_clock})
        )
        si = drain_inst.ins.sync_info
        waits = list(si.on_wait) if si is not None else []
        if len(waits) > 1:
            drain_inst.ins.sync_info = mybir.SyncInfo(
                on_wait=waits[:1], on_update=list(si.on_update)
            )
            for w in waits[1:]:
                nop = self.nc.sync.nop(nofuse=True, hint="drain_wait_split")
                nop.ins.sync_info = mybir.SyncInfo(on_wait=[w], on_update=[])
        self.nc.all_engine_barrier()
        assert self.sems is not None
        popped = self.nc._tile_sem_poison_stack.pop()
        assert popped is self._sem_poison
        self.nc.clear_and_free_semaphores(list(self.sems.allocated().values()))
        self.nc.all_engine_barrier()

    tile_mod.TileContext._drain_and_barrier = _split_drain_and_barrier
    tile_mod.TileContext._drain_split_patched = True


def _split_all_waits(nc, max_waits=1):
    """This walrus build caps sync-wait commands per instruction very low.
    Move excess waits onto same-engine NOPs inserted just before."""
    import concourse.mybir as mybir

    n = 0
    for f in nc.m.functions:
        for b in f.blocks:
            new = []
            for inst in b.instructions:
                si = getattr(inst, "sync_info", None)
                waits = list(si.on_wait) if si is not None else []
                if len(waits) > max_waits:
                    keep = waits[:max_waits]
                    extra = waits[max_waits:]
                    for w in extra:
                        nop = mybir.InstNoOp(
                            name=f"waitsplit-{n}", ins=[], outs=[]
                        )
                        n += 1
                        nop.engine = inst.engine
                        nop.sync_info = mybir.SyncInfo(
                            on_wait=[w], on_update=[]
                        )
                        new.append(nop)
                    inst.sync_info = mybir.SyncInfo(
                        on_wait=keep, on_update=list(si.on_update)
                    )
                new.append(inst)
            b.instructions[:] = new
    return n


def _segment_ids(scope):
    marks = np.zeros(N_SENT, np.int64)
    np.add.at(marks, scope[1:-1].astype(np.int64), 1)
    return np.cumsum(marks)


def _balanced_chunks(counts):
    """Partition bags into <=NCHUNK contiguous chunks, <=64 bags each,
    minimizing the max sentence count per chunk. Returns list of (b0, b1)."""
    total = int(counts.sum())

    def greedy(cap):
        bounds = []
        s = 0
        n = 0
        b0 = 0
        for b in range(N_BAGS):
            c = int(counts[b])
            if n == MAX_BAGS_PER_CHUNK or (s + c > cap and n > 0):
                bounds.append((b0, b))
                b0 = b
                s = 0
                n = 0
            s += c
            n += 1
        bounds.append((b0, N_BAGS))
        return bounds

    lo = max(int(counts.max()), total // NCHUNK)
    hi = total
    while lo < hi:
        mid = (lo + hi) // 2
        if len(greedy(mid)) <= NCHUNK:
            hi = mid
        else:
            lo = mid + 1
    bounds = greedy(lo)
    while len(bounds) < NCHUNK:
        bounds.append((N_BAGS, N_BAGS))
    return bounds


def _build_bass(Tc):
    import concourse.mybir as mybir
    from concourse import bass
    from concourse.tile import TileContext

    _patch_tile_drain()
    f32 = mybir.dt.float32
    bf16 = mybir.dt.bfloat16
    AO = mybir.AluOpType
    Act = mybir.ActivationFunctionType
    CH = CHUNKS_PER_CORE

    nc = bass.Bass("TRN2")
    d_xv = nc.dram_tensor("xv", [128, CH * Tc * 256], bf16, kind="ExternalInput")
    d_xq = nc.dram_tensor("xq", [128, CH * 2 * Tc * 128], bf16, kind="ExternalInput")
    d_mt = nc.dram_tensor("mt", [128, CH * 2 * Tc], f32, kind="ExternalInput")
    d_io = nc.dram_tensor("io", [128, 118], bf16, kind="ExternalInput")
    d_cc = nc.dram_tensor("cc", [128, 212], bf16, kind="ExternalInput")
    d_dt = nc.dram_tensor("dt", [128, 212], f32, kind="ExternalInput")
    d_bb = nc.dram_tensor("bb", [64, 53], f32, kind="ExternalInput")
    d_out = nc.dram_tensor(
        "out", [CH, 64, 53], f32, kind="ExternalOutput"
    )

    G = (Tc + EXP_GROUP - 1) // EXP_GROUP

    with TileContext(nc) as tc:
        with (
            tc.tile_pool(name="const", bufs=1) as cpool,
            tc.tile_pool(name="xvp", bufs=2) as xvp,
            tc.tile_pool(name="xqp", bufs=2) as xqp,
            tc.tile_pool(name="mtp", bufs=2) as mtp,
            tc.tile_pool(name="ep", bufs=3) as epool,
            tc.tile_pool(name="scrp", bufs=6) as scrp,
            tc.tile_pool(name="etp", bufs=8) as etp,
            tc.tile_pool(name="a2p", bufs=8) as a2pool,
            tc.tile_pool(name="miscp", bufs=3) as miscp,
            tc.tile_pool(name="ps_flt", bufs=2, space="PSUM") as ps_flt,
            tc.tile_pool(name="ps_u", bufs=2, space="PSUM") as ps_u,
            tc.tile_pool(name="ps_d", bufs=2, space="PSUM") as ps_d,
            tc.tile_pool(name="ps_o", bufs=2, space="PSUM") as ps_o,
        ):
            io = cpool.tile([128, 118], bf16, tag="io")
            cc = cpool.tile([128, 212], bf16, tag="cc")
            dt = cpool.tile([128, 212], f32, tag="dt")
            bb = cpool.tile([64, 53], f32, tag="bb")
            nc.sync.dma_start(out=io[:], in_=d_io[:])
            nc.sync.dma_start(out=cc[:], in_=d_cc[:])
            nc.sync.dma_start(out=dt[:], in_=d_dt[:])
            nc.sync.dma_start(out=bb[:], in_=d_bb[:])
            io53 = io[:, 0:53]
            io64 = io[:, 53:117]
            ones1 = io[:, 117:118]
            pending_epi = [None]

            for k in range(CH):
                xv = xvp.tile([128, Tc * 256], bf16, tag="xv")
                xq = xqp.tile([128, 2 * Tc * 128], bf16, tag="xq")
                mt = mtp.tile([128, 2 * Tc], f32, tag="mt")
                nc.sync.dma_start(
                    out=xv[:], in_=d_xv[:, k * Tc * 256 : (k + 1) * Tc * 256]
                )
                nc.sync.dma_start(
                    out=xq[:],
                    in_=d_xq[:, k * 2 * Tc * 128 : (k + 1) * 2 * Tc * 128],
                )
                nc.sync.dma_start(
                    out=mt[:], in_=d_mt[:, k * 2 * Tc : (k + 1) * 2 * Tc]
                )

                u2 = ps_u.tile([128, 256], f32, tag="u2")
                dn = ps_d.tile([64, 2], f32, tag="dn")

                flts = []
                Es = []

                def tile_tail(j, flt_g, E_g, xv=xv, mt=mt, u2=u2, dn=dn):
                    jj = j % EXP_GROUP
                    Ej = E_g[:, jj * 106 : (jj + 1) * 106]
                    et = etp.tile([128, 2], f32, tag="et")
                    scr0 = scrp.tile([128, 53], bf16, tag="scr0")
                    scr1 = scrp.tile([128, 53], bf16, tag="scr1")
                    lbl = mt[:, 2 * j : 2 * j + 1]
                    sg = mt[:, 2 * j + 1 : 2 * j + 2]
                    nc.vector.scalar_tensor_tensor(
                        scr0[:], io53, lbl, Ej[:, 0:53],
                        AO.is_equal, AO.mult, accum_out=et[:, 0:1],
                    )
                    nc.vector.scalar_tensor_tensor(
                        scr1[:], io53, lbl, Ej[:, 53:106],
                        AO.is_equal, AO.mult, accum_out=et[:, 1:2],
                    )
                    a2 = a2pool.tile([128, 128], bf16, tag="a2")
                    nc.vector.tensor_scalar(
                        a2[:, 0:64], io64, sg, et[:, 0:1], AO.is_equal, AO.mult
                    )
                    nc.gpsimd.tensor_scalar(
                        a2[:, 64:128], io64, sg, et[:, 1:2], AO.is_equal, AO.mult
                    )
                    st = j == 0
                    sp = j == Tc - 1
                    nc.tensor.matmul(
                        u2[:, 0:128], xv[:, j * 256 : j * 256 + 128], a2[:],
                        start=st, stop=sp,
                    )
                    nc.tensor.matmul(
                        u2[:, 128:256], xv[:, j * 256 + 128 : j * 256 + 256],
                        a2[:], start=st, stop=sp,
                    )
                    nc.tensor.matmul(
                        dn[:, 0:1], a2[:, 0:64], ones1, start=st, stop=sp
                    )
                    nc.tensor.matmul(
                        dn[:, 1:2], a2[:, 64:128], ones1, start=st, stop=sp
                    )

                for g in range(G):
                    j0 = g * EXP_GROUP
                    j1 = min(j0 + EXP_GROUP, Tc)
                    w = (j1 - j0) * 106
                    flt = ps_flt.tile([128, EXP_GROUP * 106], f32, tag="flt")
                    for j in range(j0, j1):
                        jj = j % EXP_GROUP
                        o = flt[:, jj * 106 : (jj + 1) * 106]
                        nc.tensor.matmul(
                            o, xq[:, (0 * Tc + j) * 128 : (0 * Tc + j + 1) * 128],
                            cc[:, 0:106], start=True, stop=False,
                        )
                        nc.tensor.matmul(
                            o, xq[:, (1 * Tc + j) * 128 : (1 * Tc + j + 1) * 128],
                            cc[:, 106:212], start=False, stop=True,
                        )
                    E = epool.tile([128, EXP_GROUP * 106], bf16, tag="E")
                    nc.scalar.activation(E[:, 0:w], flt[:, 0:w], Act.Exp)
                    flts.append(flt)
                    Es.append(E)
                    if g == 0 and pending_epi[0] is not None:
                        pending_epi[0]()
                        pending_epi[0] = None
                    if g > 0:
                        for j in range((g - 1) * EXP_GROUP,
                                       min(g * EXP_GROUP, Tc)):
                            tile_tail(j, flts[g - 1], Es[g - 1])
                for j in range((G - 1) * EXP_GROUP, Tc):
                    tile_tail(j, flts[G - 1], Es[G - 1])

                def make_epilogue(k=k, u2=u2, dn=dn):
                    def epi():
                        sden = miscp.tile([64, 2], f32, tag="sden")
                        invs = miscp.tile([64, 2], f32, tag="invs")
                        nc.vector.tensor_scalar(
                            sden[:], dn[:], 1e-30, None, AO.add
                        )
                        nc.vector.reciprocal(invs[:], sden[:])
                        rT = miscp.tile([128, 256], f32, tag="rT")
                        nc.scalar.activation(rT[:], u2[:], Act.Copy)
                        outp = ps_o.tile([64, 106], f32, tag="outp")
                        for l in range(2):
                            for hh in range(2):
                                nc.tensor.matmul(
                                    outp[:, l * 53 : (l + 1) * 53],
                                    rT[:, hh * 128 + l * 64 : hh * 128 + (l + 1) * 64],
                                    dt[:, (l * 2 + hh) * 53 : (l * 2 + hh + 1) * 53],
                                    start=(hh == 0), stop=(hh == 1),
                                )
                        t0 = miscp.tile([64, 53], f32, tag="t0")
                        outs = miscp.tile([64, 53], f32, tag="outs")
                        nc.vector.tensor_scalar(
                            t0[:], outp[:, 0:53], invs[:, 0:1], None, AO.mult
                        )
                        nc.vector.scalar_tensor_tensor(
                            outs[:], outp[:, 53:106], invs[:, 1:2], t0[:],
                            AO.mult, AO.add,
                        )
                        outs2 = miscp.tile([64, 53], f32, tag="outs2")
                        nc.vector.tensor_tensor(outs2[:], outs[:], bb[:], AO.add)
                        nc.sync.dma_start(out=d_out[k], in_=outs2[:])
                    return epi

                pending_epi[0] = make_epilogue()
            pending_epi[0]()
            pending_epi[0] = None

    _split_all_waits(nc)
    return nc


def _prep(x, rel_emb0, rel_emb1, disc, bias, relation_levels, label_index, scope):
    import concourse.mybir as mybir

    bf = mybir.dt.np(mybir.dt.bfloat16)
    CH = CHUNKS_PER_CORE
    seg = _segment_ids(np.asarray(scope))
    counts = np.bincount(seg, minlength=N_BAGS).astype(np.int64)
    cum = np.concatenate([[0], np.cumsum(counts)])
    bounds = _balanced_chunks(counts)
    max_sents = max(int(cum[b1] - cum[b0]) for b0, b1 in bounds)
    Tc = max(1, (max_sents + 127) // 128)
    Narr = CH * Tc * 128

    x = np.asarray(x, np.float32)
    labels = np.asarray(label_index, np.int64)
    xbf = x.astype(bf)

    rl = np.asarray(relation_levels, np.int64)
    c0 = np.asarray(rel_emb0, np.float32)[rl[:, 0]]
    c1 = np.asarray(rel_emb1, np.float32)[rl[:, 1]]
    ccat = np.concatenate([c0, c1], 0)  # [106, 256]
    ccT = np.ascontiguousarray(ccat.T)  # [256, 106]
    ccsb = np.zeros((128, 212), np.float32)
    ccsb[:, 0:106] = ccT[0:128]
    ccsb[:, 106:212] = ccT[128:256]

    disc = np.asarray(disc, np.float32)
    dtsb = np.zeros((128, 212), np.float32)
    for l in range(2):
        for hh in range(2):
            dtsb[:, (l * 2 + hh) * 53 : (l * 2 + hh + 1) * 53] = (
                disc[:, l * 256 + hh * 128 : l * 256 + (hh + 1) * 128].T
            )

    iosb = np.zeros((128, 118), np.float32)
    iosb[:, 0:53] = np.arange(53, dtype=np.float32)[None, :]
    iosb[:, 53:117] = np.arange(64, dtype=np.float32)[None, :]
    iosb[:, 117] = 1.0
    biasb = np.broadcast_to(np.asarray(bias, np.float32), (64, 53)).copy()

    const = {
        "io": iosb.astype(bf),
        "cc": ccsb.astype(bf),
        "dt": dtsb,
        "bb": biasb,
    }

    in_maps = []
    meta = []
    for core in range(NCORE):
        xa = np.zeros((Narr, 256), bf)
        lbl_arr = np.zeros(Narr, np.float32)
        sg_arr = np.full(Narr, 120.0, np.float32)
        cmeta = []
        for k in range(CH):
            b0, b1 = bounds[core * CH + k]
            s0, s1 = int(cum[b0]), int(cum[b1])
            L = s1 - s0
            off = k * Tc * 128
            if L > 0:
                xa[off : off + L] = xbf[s0:s1]
                lbl_arr[off : off + L] = labels[s0:s1].astype(np.float32)
                sg_arr[off : off + L] = (seg[s0:s1] - b0).astype(np.float32)
            cmeta.append((b0, b1))
        meta.append(cmeta)
        # xv[p, k*Tc*256 + j*256 + h] = xa[(k*Tc+j)*128 + p, h]
        xv = np.ascontiguousarray(
            xa.reshape(CH * Tc, 128, 256).transpose(1, 0, 2)
        ).reshape(128, CH * Tc * 256)
        # xq[p, ((k*2+h)*Tc + j)*128 + c] = xa[(k*Tc+j)*128 + c, h*128 + p]
        xq = np.ascontiguousarray(
            xa.reshape(CH, Tc, 128, 2, 128).transpose(4, 0, 3, 1, 2)
        ).reshape(128, CH * 2 * Tc * 128)
        # mt[p, k*2*Tc + j*2 + t]
        m2 = np.stack([lbl_arr, sg_arr], axis=-1)
        mtar = np.ascontiguousarray(
            m2.reshape(CH, Tc, 128, 2).transpose(2, 0, 1, 3)
        ).reshape(128, CH * 2 * Tc)
        in_maps.append({"xv": xv, "xq": xq, "mt": mtar, **const})
    return Tc, in_maps, meta


def kernel(x, rel_emb0, rel_emb1, disc, bias, relation_levels, label_index,
           scope, _trace=False):
    from concourse.bass_utils import run_bass_kernel_spmd

    Tc, in_maps, meta = _prep(
        x, rel_emb0, rel_emb1, disc, bias, relation_levels, label_index, scope
    )
    if Tc not in _CACHE:
        _CACHE[Tc] = _build_bass(Tc)
    nc = _CACHE[Tc]
    res = None
    for attempt in range(3):
        try:
            res = run_bass_kernel_spmd(
                nc, in_maps, core_ids=list(range(NCORE)), trace=_trace
            )
            break
        except Exception:
            if attempt == 2:
                raise
    out = np.zeros((N_BAGS, NCLS), np.float32)
    for core in range(NCORE):
        o = np.asarray(res.results[core]["out"])
        for k, (b0, b1) in enumerate(meta[core]):
            if b1 > b0:
                out[b0:b1] = o[k, : b1 - b0]
    kernel._last_results = res
    return out


# revision 7
# speedup vs baseline: 1.1877x; 1.1877x over previous
"""HAttentionNetwork Trainium2 kernel (v2).

Strategy (8 NeuronCores, data-parallel over bags):
- 4096 bags split into 80 contiguous chunks (10/core, <=64 bags each),
  balanced by sentence count; each chunk padded to Tc tiles of 128 sentences.
- Host pre-arranges per-core arrays so every device DMA is one big
  contiguous-row 2D slice per chunk (4 DMA instructions per chunk):
    xv [128, CH*Tc*256]  bf16   x values, tile-major (partition=sentence)
    xq [128, CH*2*Tc*128] bf16  x transposed halves (partition=hidden)
    mt [128, CH*2*Tc]    bf16   per-sentence (label, local-seg) scalars
- Per 128-sentence tile on device:
    fltT[s,c] = sum_h xq_h[h,s] * ccT_h[h,c]   (PE, 2 bf16 matmuls, [128,106])
    E = exp(fltT)  batched 4 tiles/op          (ACT, PSUM->SBUF bf16)
    et_l = sum_c (iota53==lbl) * E[:, 53l:53l+53]  (DVE scalar_tensor_tensor
                                                    with accum_out, x2)
    a2[:, 64l:64l+64] = (io64==sg) * et_l      (Pool tensor_scalar, x2)
    u2T_h += xv_j_h^T @ a2                     (PE, accumulates [hid, bagslot]
                                                -> no epilogue transpose)
    den_l += a2_l^T @ ones                     (PE, [64,1] x2)
- Chunk epilogue: rT = copy(u2T) (ACT), outp_l = rT_l @ dt_l (PE fp32),
  out = outp_0*inv(den_0) + outp_1*inv(den_1) + bias (DVE), DMA out.
Numerics: bf16 inputs, fp32 PSUM accumulation, fp32 disc projection
(same precision as the reference-validated baseline).
"""

import numpy as np

N_SENT = 262144
N_BAGS = 4096
HIDDEN = 256
L0 = 14
NCLS = 53
NCORE = 8
CHUNKS_PER_CORE = 10
NCHUNK = NCORE * CHUNKS_PER_CORE
MAX_BAGS_PER_CHUNK = 64
EXP_GROUP = 4

_CACHE = {}


def _patch_tile_drain():
    # This walrus build rejects Drain instructions carrying more than ~1 sync
    # wait. Split the Tile final-drain waits across SP nops, one wait each.
    import concourse.mybir as mybir
    import concourse.tile as tile_mod
    from concourse.vector_clock import ScopedClock

    if getattr(tile_mod.TileContext, "_drain_split_patched", False):
        return

    def _split_drain_and_barrier(self, tick_clock, wait_clock):
        drain_inst = self.nc.sync.drain()
        wait_clock.add_sem_waits(
            drain_inst.ins, ScopedClock({None: tick_clock.global_clock})
        )
        si = drain_inst.ins.sync_info
        waits = list(si.on_wait) if si is not None else []
        if len(waits) > 1:
            drain_inst.ins.sync_info = mybir.SyncInfo(
                on_wait=waits[:1], on_update=list(si.on_update)
            )
            for w in waits[1:]:
                nop = self.nc.sync.nop(nofuse=True, hint="drain_wait_split")
                nop.ins.sync_info = mybir.SyncInfo(on_wait=[w], on_update=[])
        self.nc.all_engine_barrier()
        assert self.sems is not None
        popped = self.nc._tile_sem_poison_stack.pop()
        assert popped is self._sem_poison
        self.nc.clear_and_free_semaphores(list(self.sems.allocated().values()))
        self.nc.all_engine_barrier()

    tile_mod.TileContext._drain_and_barrier = _split_drain_and_barrier
    tile_mod.TileContext._drain_split_patched = True


def _split_all_waits(nc, max_waits=1):
    """This walrus build caps sync-wait commands per instruction very low.
    Move excess waits onto same-engine NOPs inserted just before."""
    import concourse.mybir as mybir

    n = 0
    for f in nc.m.functions:
        for b in f.blocks:
            new = []
            for inst in b.instructions:
                si = getattr(inst, "sync_info", None)
                waits = list(si.on_wait) if si is not None else []
                if len(waits) > max_waits:
                    keep = waits[:max_waits]
                    extra = waits[max_waits:]
                    for w in extra:
                        nop = mybir.InstNoOp(
                            name=f"waitsplit-{n}", ins=[], outs=[]
                        )
                        n += 1
                        nop.engine = inst.engine
                        nop.sync_info = mybir.SyncInfo(
                            on_wait=[w], on_update=[]
                        )
                        new.append(nop)
                    inst.sync_info = mybir.SyncInfo(
                        on_wait=keep, on_update=list(si.on_update)
                    )
                new.append(inst)
            b.instructions[:] = new
    return n


def _segment_ids(scope):
    marks = np.zeros(N_SENT, np.int64)
    np.add.at(marks, scope[1:-1].astype(np.int64), 1)
    return np.cumsum(marks)


def _balanced_chunks(counts):
    """Partition bags into <=NCHUNK contiguous chunks, <=64 bags each,
    minimizing the max sentence count per chunk. Returns list of (b0, b1)."""
    total = int(counts.sum())

    def greedy(cap):
        bounds = []
        s = 0
        n = 0
        b0 = 0
        for b in range(N_BAGS):
            c = int(counts[b])
            if n == MAX_BAGS_PER_CHUNK or (s + c > cap and n > 0):
                bounds.append((b0, b))
                b0 = b
                s = 0
                n = 0
            s += c
            n += 1
        bounds.append((b0, N_BAGS))
        return bounds

    lo = max(int(counts.max()), total // NCHUNK)
    hi = total
    while lo < hi:
        mid = (lo + hi) // 2
        if len(greedy(mid)) <= NCHUNK:
            hi = mid
        else:
            lo = mid + 1
    bounds = greedy(lo)
    while len(bounds) < NCHUNK:
        bounds.append((N_BAGS, N_BAGS))
    return bounds


def _build_bass(Tc):
    import concourse.mybir as mybir
    from concourse import bass
    from concourse.tile import TileContext

    _patch_tile_drain()
    f32 = mybir.dt.float32
    bf16 = mybir.dt.bfloat16
    AO = mybir.AluOpType
    Act = mybir.ActivationFunctionType
    CH = CHUNKS_PER_CORE

    nc = bass.Bass("TRN2")
    d_xv = nc.dram_tensor("xv", [128, CH * Tc * 256], bf16, kind="ExternalInput")
    d_xq = nc.dram_tensor("xq", [128, CH * 2 * Tc * 128], bf16, kind="ExternalInput")
    d_mt = nc.dram_tensor("mt", [128, CH * 2 * Tc], f32, kind="ExternalInput")
    d_io = nc.dram_tensor("io", [128, 118], bf16, kind="ExternalInput")
    d_cc = nc.dram_tensor("cc", [128, 212], bf16, kind="ExternalInput")
    d_dt = nc.dram_tensor("dt", [128, 212], f32, kind="ExternalInput")
    d_bb = nc.dram_tensor("bb", [64, 53], f32, kind="ExternalInput")
    d_out = nc.dram_tensor(
        "out", [CH, 64, 53], f32, kind="ExternalOutput"
    )

    G = (Tc + EXP_GROUP - 1) // EXP_GROUP

    with TileContext(nc) as tc:
        with (
            tc.tile_pool(name="const", bufs=1) as cpool,
            tc.tile_pool(name="xvp", bufs=2) as xvp,
            tc.tile_pool(name="xqp", bufs=2) as xqp,
            tc.tile_pool(name="mtp", bufs=2) as mtp,
            tc.tile_pool(name="ep", bufs=3) as epool,
            tc.tile_pool(name="scrp", bufs=6) as scrp,
            tc.tile_pool(name="etp", bufs=8) as etp,
            tc.tile_pool(name="a2p", bufs=8) as a2pool,
            tc.tile_pool(name="miscp", bufs=3) as miscp,
            tc.tile_pool(name="ps_flt", bufs=2, space="PSUM") as ps_flt,
            tc.tile_pool(name="ps_ua", bufs=1, space="PSUM") as ps_ua,
            tc.tile_pool(name="ps_ub", bufs=1, space="PSUM") as ps_ub,
            tc.tile_pool(name="ps_d0", bufs=1, space="PSUM") as ps_d0,
            tc.tile_pool(name="ps_d1", bufs=1, space="PSUM") as ps_d1,
            tc.tile_pool(name="ps_o", bufs=2, space="PSUM") as ps_o,
        ):
            io = cpool.tile([128, 118], bf16, tag="io")
            cc = cpool.tile([128, 212], bf16, tag="cc")
            dt = cpool.tile([128, 212], f32, tag="dt")
            bb = cpool.tile([64, 53], f32, tag="bb")
            nc.sync.dma_start(out=io[:], in_=d_io[:])
            nc.sync.dma_start(out=cc[:], in_=d_cc[:])
            nc.sync.dma_start(out=dt[:], in_=d_dt[:])
            nc.sync.dma_start(out=bb[:], in_=d_bb[:])
            io53 = io[:, 0:53]
            io64 = io[:, 53:117]
            ones1 = io[:, 117:118]
            pending_epi = [None]

            for k in range(CH):
                xv = xvp.tile([128, Tc * 256], bf16, tag="xv")
                xq = xqp.tile([128, 2 * Tc * 128], bf16, tag="xq")
                mt = mtp.tile([128, 2 * Tc], f32, tag="mt")
                nc.sync.dma_start(
                    out=xv[:], in_=d_xv[:, k * Tc * 256 : (k + 1) * Tc * 256]
                )
                nc.sync.dma_start(
                    out=xq[:],
                    in_=d_xq[:, k * 2 * Tc * 128 : (k + 1) * 2 * Tc * 128],
                )
                nc.sync.dma_start(
                    out=mt[:], in_=d_mt[:, k * 2 * Tc : (k + 1) * 2 * Tc]
                )

                u2a = ps_ua.tile([128, 128], f32, tag="u2a")
                u2b = ps_ub.tile([128, 128], f32, tag="u2b")
                dn0 = ps_d0.tile([64, 1], f32, tag="dn0")
                dn1 = ps_d1.tile([64, 1], f32, tag="dn1")

                flts = []
                Es = []

                def tile_tail(j, flt_g, E_g, xv=xv, mt=mt, u2a=u2a, u2b=u2b, dn0=dn0, dn1=dn1):
                    jj = j % EXP_GROUP
                    Ej = E_g[:, jj * 106 : (jj + 1) * 106]
                    et = etp.tile([128, 2], f32, tag="et")
                    scr0 = scrp.tile([128, 53], bf16, tag="scr0")
                    scr1 = scrp.tile([128, 53], bf16, tag="scr1")
                    lbl = mt[:, 2 * j : 2 * j + 1]
                    sg = mt[:, 2 * j + 1 : 2 * j + 2]
                    nc.vector.scalar_tensor_tensor(
                        scr0[:], io53, lbl, Ej[:, 0:53],
                        AO.is_equal, AO.mult, accum_out=et[:, 0:1],
                    )
                    nc.vector.scalar_tensor_tensor(
                        scr1[:], io53, lbl, Ej[:, 53:106],
                        AO.is_equal, AO.mult, accum_out=et[:, 1:2],
                    )
                    a2 = a2pool.tile([128, 128], bf16, tag="a2")
                    nc.vector.tensor_scalar(
                        a2[:, 0:64], io64, sg, et[:, 0:1], AO.is_equal, AO.mult
                    )
                    nc.gpsimd.tensor_scalar(
                        a2[:, 64:128], io64, sg, et[:, 1:2], AO.is_equal, AO.mult
                    )
                    st = j == 0
                    sp = j == Tc - 1
                    nc.tensor.matmul(
                        u2a[:], xv[:, j * 256 : j * 256 + 128], a2[:],
                        start=st, stop=sp,
                    )
                    nc.tensor.matmul(
                        u2b[:], xv[:, j * 256 + 128 : j * 256 + 256],
                        a2[:], start=st, stop=sp,
                    )
                    nc.tensor.matmul(
                        dn0[:], a2[:, 0:64], ones1, start=st, stop=sp
                    )
                    nc.tensor.matmul(
                        dn1[:], a2[:, 64:128], ones1, start=st, stop=sp
                    )

                for g in range(G):
                    j0 = g * EXP_GROUP
                    j1 = min(j0 + EXP_GROUP, Tc)
                    w = (j1 - j0) * 106
                    flt = ps_flt.tile([128, EXP_GROUP * 106], f32, tag="flt")
                    for j in range(j0, j1):
                        jj = j % EXP_GROUP
                        o = flt[:, jj * 106 : (jj + 1) * 106]
                        nc.tensor.matmul(
                            o, xq[:, (0 * Tc + j) * 128 : (0 * Tc + j + 1) * 128],
                            cc[:, 0:106], start=True, stop=False,
                        )
                        nc.tensor.matmul(
                            o, xq[:, (1 * Tc + j) * 128 : (1 * Tc + j + 1) * 128],
                            cc[:, 106:212], start=False, stop=True,
                        )
                    E = epool.tile([128, EXP_GROUP * 106], bf16, tag="E")
                    nc.scalar.activation(E[:, 0:w], flt[:, 0:w], Act.Exp)
                    flts.append(flt)
                    Es.append(E)
                    if g == 0 and pending_epi[0] is not None:
                        pending_epi[0]()
                        pending_epi[0] = None
                    if g > 0:
                        for j in range((g - 1) * EXP_GROUP,
                                       min(g * EXP_GROUP, Tc)):
                            tile_tail(j, flts[g - 1], Es[g - 1])
                for j in range((G - 1) * EXP_GROUP, Tc):
                    tile_tail(j, flts[G - 1], Es[G - 1])

                def make_epilogue(k=k, u2a=u2a, u2b=u2b, dn0=dn0, dn1=dn1):
                    def epi():
                        sden = miscp.tile([64, 2], f32, tag="sden")
                        invs = miscp.tile([64, 2], f32, tag="invs")
                        nc.vector.tensor_scalar(
                            sden[:, 0:1], dn0[:], 1e-30, None, AO.add
                        )
                        nc.vector.tensor_scalar(
                            sden[:, 1:2], dn1[:], 1e-30, None, AO.add
                        )
                        nc.vector.reciprocal(invs[:], sden[:])
                        rT = miscp.tile([128, 256], f32, tag="rT")
                        nc.scalar.activation(rT[:, 0:128], u2a[:], Act.Copy)
                        nc.scalar.activation(rT[:, 128:256], u2b[:], Act.Copy)
                        outp = ps_o.tile([64, 106], f32, tag="outp")
                        for l in range(2):
                            for hh in range(2):
                                nc.tensor.matmul(
                                    outp[:, l * 53 : (l + 1) * 53],
                                    rT[:, hh * 128 + l * 64 : hh * 128 + (l + 1) * 64],
                                    dt[:, (l * 2 + hh) * 53 : (l * 2 + hh + 1) * 53],
                                    start=(hh == 0), stop=(hh == 1),
                                )
                        t0 = miscp.tile([64, 53], f32, tag="t0")
                        outs = miscp.tile([64, 53], f32, tag="outs")
                        nc.vector.tensor_scalar(
                            t0[:], outp[:, 0:53], invs[:, 0:1], None, AO.mult
                        )
                        nc.vector.scalar_tensor_tensor(
                            outs[:], outp[:, 53:106], invs[:, 1:2], t0[:],
                            AO.mult, AO.add,
                        )
                        outs2 = miscp.tile([64, 53], f32, tag="outs2")
                        nc.vector.tensor_tensor(outs2[:], outs[:], bb[:], AO.add)
                        nc.scalar.dma_start(out=d_out[k], in_=outs2[:])
                    return epi

                pending_epi[0] = make_epilogue()
            pending_epi[0]()
            pending_epi[0] = None

    _split_all_waits(nc)
    return nc


def _prep(x, rel_emb0, rel_emb1, disc, bias, relation_levels, label_index, scope):
    import concourse.mybir as mybir

    bf = mybir.dt.np(mybir.dt.bfloat16)
    CH = CHUNKS_PER_CORE
    seg = _segment_ids(np.asarray(scope))
    counts = np.bincount(seg, minlength=N_BAGS).astype(np.int64)
    cum = np.concatenate([[0], np.cumsum(counts)])
    bounds = _balanced_chunks(counts)
    max_sents = max(int(cum[b1] - cum[b0]) for b0, b1 in bounds)
    Tc = max(1, (max_sents + 127) // 128)
    Narr = CH * Tc * 128

    x = np.asarray(x, np.float32)
    labels = np.asarray(label_index, np.int64)
    xbf = x.astype(bf)

    rl = np.asarray(relation_levels, np.int64)
    c0 = np.asarray(rel_emb0, np.float32)[rl[:, 0]]
    c1 = np.asarray(rel_emb1, np.float32)[rl[:, 1]]
    ccat = np.concatenate([c0, c1], 0)  # [106, 256]
    ccT = np.ascontiguousarray(ccat.T)  # [256, 106]
    ccsb = np.zeros((128, 212), np.float32)
    ccsb[:, 0:106] = ccT[0:128]
    ccsb[:, 106:212] = ccT[128:256]

    disc = np.asarray(disc, np.float32)
    dtsb = np.zeros((128, 212), np.float32)
    for l in range(2):
        for hh in range(2):
            dtsb[:, (l * 2 + hh) * 53 : (l * 2 + hh + 1) * 53] = (
                disc[:, l * 256 + hh * 128 : l * 256 + (hh + 1) * 128].T
            )

    iosb = np.zeros((128, 118), np.float32)
    iosb[:, 0:53] = np.arange(53, dtype=np.float32)[None, :]
    iosb[:, 53:117] = np.arange(64, dtype=np.float32)[None, :]
    iosb[:, 117] = 1.0
    biasb = np.broadcast_to(np.asarray(bias, np.float32), (64, 53)).copy()

    const = {
        "io": iosb.astype(bf),
        "cc": ccsb.astype(bf),
        "dt": dtsb,
        "bb": biasb,
    }

    in_maps = []
    meta = []
    for core in range(NCORE):
        xa = np.zeros((Narr, 256), bf)
        lbl_arr = np.zeros(Narr, np.float32)
        sg_arr = np.full(Narr, 120.0, np.float32)
        cmeta = []
        for k in range(CH):
            b0, b1 = bounds[core * CH + k]
            s0, s1 = int(cum[b0]), int(cum[b1])
            L = s1 - s0
            off = k * Tc * 128
            if L > 0:
                xa[off : off + L] = xbf[s0:s1]
                lbl_arr[off : off + L] = labels[s0:s1].astype(np.float32)
                sg_arr[off : off + L] = (seg[s0:s1] - b0).astype(np.float32)
            cmeta.append((b0, b1))
        meta.append(cmeta)
        # xv[p, k*Tc*256 + j*256 + h] = xa[(k*Tc+j)*128 + p, h]
        xv = np.ascontiguousarray(
            xa.reshape(CH * Tc, 128, 256).transpose(1, 0, 2)
        ).reshape(128, CH * Tc * 256)
        # xq[p, ((k*2+h)*Tc + j)*128 + c] = xa[(k*Tc+j)*128 + c, h*128 + p]
        xq = np.ascontiguousarray(
            xa.reshape(CH, Tc, 128, 2, 128).transpose(4, 0, 3, 1, 2)
        ).reshape(128, CH * 2 * Tc * 128)
        # mt[p, k*2*Tc + j*2 + t]
        m2 = np.stack([lbl_arr, sg_arr], axis=-1)
        mtar = np.ascontiguousarray(
            m2.reshape(CH, Tc, 128, 2).transpose(2, 0, 1, 3)
        ).reshape(128, CH * 2 * Tc)
        in_maps.append({"xv": xv, "xq": xq, "mt": mtar, **const})
    return Tc, in_maps, meta


def kernel(x, rel_emb0, rel_emb1, disc, bias, relation_levels, label_index,
           scope, _trace=False):
    from concourse.bass_utils import run_bass_kernel_spmd

    Tc, in_maps, meta = _prep(
        x, rel_emb0, rel_emb1, disc, bias, relation_levels, label_index, scope
    )
    if Tc not in _CACHE:
        _CACHE[Tc] = _build_bass(Tc)
    nc = _CACHE[Tc]
    res = None
    for attempt in range(3):
        try:
            res = run_bass_kernel_spmd(
                nc, in_maps, core_ids=list(range(NCORE)), trace=_trace
            )
            break
        except Exception:
            if attempt == 2:
                raise
    out = np.zeros((N_BAGS, NCLS), np.float32)
    for core in range(NCORE):
        o = np.asarray(res.results[core]["out"])
        for k, (b0, b1) in enumerate(meta[core]):
            if b1 > b0:
                out[b0:b1] = o[k, : b1 - b0]
    kernel._last_results = res
    return out
